# revision 46
# baseline (speedup 1.0000x reference)
"""Trainium2 Bass kernel for the pose-estimation loss (pm / t_center / t_depth).

Strategy
--------
pm[n] = mean_p | (pred_R[n]-gt_R[n]) @ obj_points[obj_id[n], p] |_1 / diam[obj_id[n]]

The data-dependent gather obj_points[obj_id] is folded into the matmul:
    Y[(i,n), p] = sum_{o,j} A[(o,j),(i,n)] * B[(o,j), p]
with A[(o,j),(i,n)] = [obj_id[n]==o] * dR[n,i,j]   (24 x 384, built on host)
     B[(o,j), p]    = obj_points[o, p, j]          (24 x 12500 per core)

The bottleneck is draining PSUM through abs+sum.  Only DVE and ACT can read
PSUM, each at 1 elem/lane/cycle (a DVE op may read at most ONE non-scalar
input from PSUM, so no 2-stream tricks).  Measured drain rates: DVE
tensor_reduce(abs) ~115 G elem/s, ACT activation(Abs, accum_out) ~99 G
elem/s; both run flat-out on disjoint PSUM bank pairs, fully fused into
per-instruction accumulator columns.

This version is RAW bass (no TileContext): the whole pipeline is a static
double-buffered schedule synchronized with 8 hand-placed counter
semaphores.  Tile's scheduler allocates ~250 dependency semaphores for
this program and spends ~10us tearing them down inside the measured
window; the manual schedule eliminates that entirely.

Engine program (per core):
  sync ring : A piece | B[512:1536] | B[2560:3312] | out
  scalar ring: B[0:512] | tsite | B[1536:2560]
  Tensor    : per chunk 2 matmuls (q0/q1 -> DVE tile, q2/q3 -> ACT tile),
              4 PE row-group quadrants, K=24, cold-clock 1.2 GHz (HAM never
              engages on this part; even cold the PE outruns the drains).
  Vector    : t_site losses early, then tensor_reduce(abs) per DVE tile,
              final acc-column sum -> out_sb.
  Scalar    : Abs-table warmup, then activation(Abs, accum_out) per ACT tile.

Per core output: out[128, 3] = [pm partial sum, t_center, t_depth].
Host: pm = sum_over_cores(out[:,0]) / 100000 / diam[obj_id].
"""

import os
import sys

import numpy as np

os.environ.setdefault("MYCRO_LOCAL_CACHE", "1")
if "/opt/trn_rl_repo" not in sys.path:
    sys.path.insert(0, "/opt/trn_rl_repo")

# ---- problem constants (hardcoded, must match the reference) ----
N_SAMPLES = 128
NUM_OBJECTS = 8
NUM_POINTS = 100000
N_CORES = 8

PTS_PER_CORE = NUM_POINTS // N_CORES  # 12500
ICHUNKS = 3                           # (i) coordinate chunks of 128 samples
A_COLS = ICHUNKS * 128                # 384

# DVE quadrants (q0, q1) and ACT quadrants (q2, q3): column counts matched
# to the measured drain rates (both engines finish together).
DVE_CHUNKS = [512] * 6 + [152]
DVE_COLS = sum(DVE_CHUNKS)            # 3224
ACT_CHUNKS = [512] * 5 + [466]
ACT_COLS = sum(ACT_CHUNKS)            # 3026
assert 2 * DVE_COLS + 2 * ACT_COLS == PTS_PER_CORE

AB_COLS = A_COLS + DVE_COLS           # 3696 (q2/q3 rows zero-padded at the end)
N_ACC = ICHUNKS * (len(DVE_CHUNKS) + len(ACT_CHUNKS))  # 39 accum columns

# B-piece index covering each 512-col chunk (pieces: 0 = cols 0:512,
# 1 = 512:1536, 2 = 1536:2560, 3 = 2560:3312).
_CHUNK_PIECE = {0: 0, 1: 1, 2: 1, 3: 2, 4: 2, 5: 3, 6: 3}

_CACHE = {}


def _build_module():
    """Build + compile the single-core Bass program (same program on all cores)."""
    if "nc" in _CACHE:
        return _CACHE["nc"]

    import concourse.bass as bass  # noqa: F401  (import registers engines)
    from concourse import bacc, mybir

    f32 = mybir.dt.float32
    bf16 = mybir.dt.bfloat16

    # detect_race_conditions=False: the checker has no notion of same-engine
    # FIFO order (a hardware guarantee) and flags every same-engine
    # write->read chain in a raw-bass program.  Cross-engine ordering is
    # fully covered by the explicit semaphores below.
    nc = bacc.Bacc("TRN2", target_bir_lowering=False, debug=False,
                   detect_race_conditions=False)

    abmat = nc.dram_tensor("abmat", [128, AB_COLS], bf16, kind="ExternalInput").ap()
    tsite = nc.dram_tensor("tsite", [128, 6], f32, kind="ExternalInput").ap()
    out = nc.dram_tensor("out", [128, 3], f32, kind="ExternalOutput").ap()

    ab_sb = nc.alloc_sbuf_tensor("ab_sb", [128, AB_COLS], bf16).ap()
    ts_sb = nc.alloc_sbuf_tensor("ts_sb", [128, 6], f32).ap()
    acc = nc.alloc_sbuf_tensor("acc", [128, N_ACC], f32).ap()
    asum = nc.alloc_sbuf_tensor("asum", [128, 2, 512], bf16).ap()
    out_sb = nc.alloc_sbuf_tensor("out_sb", [128, 3], f32).ap()
    d_sb = nc.alloc_sbuf_tensor("d_sb", [128, 3], f32).ap()

    a_sb = ab_sb[:, 0:A_COLS]
    b_sb = ab_sb[:, A_COLS:]

    # PSUM: DVE tiles in banks 0-3 (two [128,2,512] buffers), ACT tiles in
    # banks 4-7.
    v_ps = [nc.place_psum_tensor(f"v_ps{b}", [128, 2, 512], f32, bank=2 * b).ap()
            for b in range(2)]
    t_ps = [nc.place_psum_tensor(f"t_ps{b}", [128, 2, 512], f32, bank=4 + 2 * b).ap()
            for b in range(2)]

    # Counter semaphores (cleared below before any engine waits on them).
    # One per DMA piece: per-engine completion increments from different
    # pieces on one ring can interleave, so shared-counter thresholds are
    # unsound.
    s_da = nc.alloc_semaphore("s_da")         # A piece, i=0 cols (x16)
    s_da2 = nc.alloc_semaphore("s_da2")       # A piece, i=1,2 cols (x16)
    s_db0v = nc.alloc_semaphore("s_db0v")     # B cols 0:512, DVE rows (x16)
    s_db0a = nc.alloc_semaphore("s_db0a")     # B cols 0:512, ACT rows (x16)
    s_db = [nc.alloc_semaphore(f"s_db{p}") for p in range(4)]  # B pieces (x16)
    s_dts = nc.alloc_semaphore("s_dts")       # tsite piece (x16)
    s_mmv = nc.alloc_semaphore("s_mmv")       # DVE-path chunks matmul'd
    s_mma = nc.alloc_semaphore("s_mma")       # ACT-path chunks matmul'd
    s_v = nc.alloc_semaphore("s_v")           # DVE tiles drained
    s_a = nc.alloc_semaphore("s_a")           # ACT tiles drained
    s_fin = nc.alloc_semaphore("s_fin")       # ACT accum columns all landed
    s_out = nc.alloc_semaphore("s_out")       # out_sb complete
    s_odma = nc.alloc_semaphore("s_odma")     # out DMA completion (x16)
    all_sems = [s_da, s_da2, s_db0v, s_db0a, *s_db, s_dts, s_mmv, s_mma, s_v,
                s_a, s_fin, s_out, s_odma]

    # DMAs first: their completion increments land multi-us later, safely
    # after the gpsimd sem clears below.  The first pieces are minimal (A
    # for i=0 only; B cols 0:512 split by engine-path rows) because their
    # ~2us completion round-trip gates the first matmuls.
    nc.sync.dma_start(out=ab_sb[:, 0:128],
                      in_=abmat[:, 0:128]).then_inc(s_da, 16)
    nc.scalar.dma_start(out=ab_sb[0:56, A_COLS : A_COLS + 512],
                        in_=abmat[0:56, A_COLS : A_COLS + 512]).then_inc(s_db0v, 16)
    nc.sync.dma_start(out=ab_sb[:, 128:A_COLS],
                      in_=abmat[:, 128:A_COLS]).then_inc(s_da2, 16)
    nc.scalar.dma_start(out=ab_sb[64:120, A_COLS : A_COLS + 512],
                        in_=abmat[64:120, A_COLS : A_COLS + 512]).then_inc(s_db0a, 16)
    nc.sync.dma_start(out=ab_sb[:, A_COLS + 512 : A_COLS + 1536],
                      in_=abmat[:, A_COLS + 512 : A_COLS + 1536]).then_inc(s_db[1], 16)
    nc.scalar.dma_start(out=ts_sb, in_=tsite).then_inc(s_dts, 16)
    nc.sync.dma_start(out=ab_sb[:, A_COLS + 2560 :],
                      in_=abmat[:, A_COLS + 2560 :]).then_inc(s_db[3], 16)
    nc.scalar.dma_start(out=ab_sb[:, A_COLS + 1536 : A_COLS + 2560],
                        in_=abmat[:, A_COLS + 1536 : A_COLS + 2560]).then_inc(s_db[2], 16)

    # Clear our semaphores (stale values survive across NEFF executions),
    # then barrier so no engine's sem-wait can read a stale value.  The DMA
    # completion increments race this clear only in theory — they land
    # >2us after the clears retire.
    for s in all_sems:
        nc.gpsimd.sem_clear(s)
    nc.all_engine_barrier()

    # No ACT warm-up needed: bacc auto-inserts the Abs ACT_TABLE_LOAD right
    # before the first ACTIVATE in the queue, and the sem wait fuses onto
    # the ACTIVATE itself, so the ~1.3us table load runs while DMAs stream.

    # ---- main pipeline ----
    # Global chunk order: for each i-chunk interleave v0 a0 v1 a1 ... v6.
    jv = 0  # DVE-path chunk counter
    ja = 0  # ACT-path chunk counter
    col = 0
    vcols = []  # acc columns owned by DVE (their sum happens in the final
    acols = []  # reduce regardless; recorded only for clarity)
    for i in range(ICHUNKS):
        ai = slice(i * 128, (i + 1) * 128)
        order = []
        for k in range(len(DVE_CHUNKS)):
            order.append(("v", k))
            if k < len(ACT_CHUNKS):
                order.append(("a", k))
        s_ai = s_da if i == 0 else s_da2
        for kind, k in order:
            off = 512 * k
            if kind == "v":
                w = DVE_CHUNKS[k]
                ps = v_ps[jv % 2]
                s_piece = s_db0v if k == 0 else s_db[_CHUNK_PIECE[k]]
                nc.tensor.wait_ge(s_ai, 16)
                nc.tensor.wait_ge(s_piece, 16)
                if jv >= 2:
                    nc.tensor.wait_ge(s_v, jv - 1)
                nc.tensor.matmul(
                    ps[:, 0, 0:w], lhsT=a_sb[0:24, ai],
                    rhs=b_sb[0:24, off : off + w],
                    start=True, stop=True, tile_position=(0, 0),
                )
                nc.tensor.matmul(
                    ps[:, 1, 0:w], lhsT=a_sb[32:56, ai],
                    rhs=b_sb[32:56, off : off + w],
                    start=True, stop=True, tile_position=(32, 0),
                ).then_inc(s_mmv, 1)
                nc.vector.wait_ge(s_mmv, jv + 1)
                nc.vector.tensor_reduce(
                    out=acc[:, col : col + 1], in_=ps[:, :, 0:w],
                    axis=mybir.AxisListType.XY, op=mybir.AluOpType.add,
                    apply_absolute_value=True,
                ).then_inc(s_v, 1)
                jv += 1
                vcols.append(col)
            else:
                w = ACT_CHUNKS[k]
                ps = t_ps[ja % 2]
                s_piece = s_db0a if k == 0 else s_db[_CHUNK_PIECE[k]]
                nc.tensor.wait_ge(s_ai, 16)
                nc.tensor.wait_ge(s_piece, 16)
                if ja >= 2:
                    nc.tensor.wait_ge(s_a, ja - 1)
                nc.tensor.matmul(
                    ps[:, 0, 0:w], lhsT=a_sb[64:88, ai],
                    rhs=b_sb[64:88, off : off + w],
                    start=True, stop=True, tile_position=(64, 0),
                )
                nc.tensor.matmul(
                    ps[:, 1, 0:w], lhsT=a_sb[96:120, ai],
                    rhs=b_sb[96:120, off : off + w],
                    start=True, stop=True, tile_position=(96, 0),
                ).then_inc(s_mma, 1)
                nc.scalar.wait_ge(s_mma, ja + 1)
                nc.scalar.activation(
                    out=asum[:, :, 0:w], in_=ps[:, :, 0:w],
                    func=mybir.ActivationFunctionType.Abs,
                    accum_out=acc[:, col : col + 1],
                ).then_inc(s_a, 1)
                ja += 1
                acols.append(col)
            col += 1
    assert jv == ICHUNKS * len(DVE_CHUNKS) and ja == ICHUNKS * len(ACT_CHUNKS)

    # ACT signals its accumulator columns are all written (queue-ordered
    # after the last ACTIVATION_READ_ACCUMULATOR).
    nc.scalar.nop(nofuse=True).then_inc(s_fin, 1)

    # t_site losses, placed after the drain loop so they never delay the
    # first chunk drains; they run in the shadow of ACT's remaining chunks:
    # d = gt - pred; t_center = |d0|+|d1|; t_depth = |d2|.
    nc.vector.wait_ge(s_dts, 16)
    nc.vector.tensor_sub(d_sb, ts_sb[:, 0:3], ts_sb[:, 3:6])
    # Explicit pipe drain: raw bass does not auto-insert the DVE DRAIN, and
    # the next op would read d_sb while the sub's writes are still in the
    # 8-stage pipe.
    nc.vector.drain()
    nc.vector.tensor_reduce(
        out=out_sb[:, 1:2], in_=d_sb[:, 0:2], axis=mybir.AxisListType.X,
        op=mybir.AluOpType.add, apply_absolute_value=True,
    )
    nc.vector.tensor_reduce(
        out=out_sb[:, 2:3], in_=d_sb[:, 2:3], axis=mybir.AxisListType.X,
        op=mybir.AluOpType.add, apply_absolute_value=True,
    )

    # Final: pm partial = sum of all accumulator columns.  Drain first: the
    # last DVE chunk-reduce wrote its acc column on this same engine and raw
    # bass does not auto-insert the DVE pipe DRAIN.
    nc.vector.wait_ge(s_fin, 1)
    nc.vector.drain()
    nc.vector.tensor_reduce(
        out=out_sb[:, 0:1], in_=acc[:, 0:col], axis=mybir.AxisListType.X,
        op=mybir.AluOpType.add,
    ).then_inc(s_out, 1)

    # No final barrier: each engine halts when its queue ends, so the NEFF
    # packager's per-engine postamble (a ~6us semaphore sweep) overlaps the
    # other engines' remaining work instead of starting after a global
    # barrier.  The runtime fences the in-flight output DMA at NEFF end.
    nc.sync.wait_ge(s_out, 1)
    nc.sync.dma_start(out=out, in_=out_sb).then_inc(s_odma, 16)

    nc.compile()
    _CACHE["nc"] = nc
    return nc


def _prepare_in_maps(obj_id, gt_cam_R_m2c, pred_cam_R_m2c, gt_cam_t_m2c_site,
                     pred_cam_t_m2c_site, obj_points, obj_diameters):
    obj_id = np.asarray(obj_id).astype(np.int64)
    dR = (np.asarray(pred_cam_R_m2c, np.float32)
          - np.asarray(gt_cam_R_m2c, np.float32))          # [N, 3, 3] (i, j)
    pts = np.asarray(obj_points, np.float32)               # [8, P, 3]

    import ml_dtypes

    # A[(o,j), (i,n)] = [obj_id[n]==o] * dR[n, i, j]
    afull = np.zeros((NUM_OBJECTS, 3, 3, N_SAMPLES), np.float32)  # [o, j, i, n]
    afull[obj_id, :, :, np.arange(N_SAMPLES)] = dR.transpose(0, 2, 1)  # [n, j, i]
    a24 = afull.reshape(NUM_OBJECTS * 3, 3 * N_SAMPLES)    # rows (o,j), cols i*128+n

    # B rows (o,j), cols p
    b24 = pts.transpose(0, 2, 1).reshape(NUM_OBJECTS * 3, NUM_POINTS)

    ts_host = np.concatenate(
        [np.asarray(gt_cam_t_m2c_site, np.float32),
         np.asarray(pred_cam_t_m2c_site, np.float32)], axis=1)  # [128, 6]

    quad_cols = [DVE_COLS, DVE_COLS, ACT_COLS, ACT_COLS]
    in_maps = []
    for c in range(N_CORES):
        cols = b24[:, c * PTS_PER_CORE : (c + 1) * PTS_PER_CORE]
        ab = np.zeros((128, AB_COLS), np.float32)
        off = 0
        for g in range(4):
            w = quad_cols[g]
            ab[32 * g : 32 * g + 24, 0:A_COLS] = a24
            ab[32 * g : 32 * g + 24, A_COLS : A_COLS + w] = cols[:, off : off + w]
            off += w
        in_maps.append({
            "abmat": np.ascontiguousarray(ab).astype(ml_dtypes.bfloat16),
            "tsite": ts_host,
        })
    return in_maps, obj_id, np.asarray(obj_diameters, np.float32)


def _postprocess(results, obj_id, obj_diameters):
    pm_sum = np.zeros(N_SAMPLES, np.float64)
    for c in range(N_CORES):
        pm_sum += results[c]["out"][:, 0].astype(np.float64)
    pm = (pm_sum / NUM_POINTS / obj_diameters[obj_id].astype(np.float64)).astype(
        np.float32)
    t_center = results[0]["out"][:, 1].astype(np.float32)
    t_depth = results[0]["out"][:, 2].astype(np.float32)
    return pm, t_center, t_depth


def run(inputs, trace=False):
    """Run on the 8 NeuronCores. Returns ((pm, t_center, t_depth), BassKernelResults)."""
    from concourse.bass_utils import run_bass_kernel_spmd

    nc = _build_module()
    in_maps, obj_id, diam = _prepare_in_maps(**inputs)
    res = run_bass_kernel_spmd(nc, in_maps, list(range(N_CORES)), trace=trace)
    return _postprocess(res.results, obj_id, diam), res


def run_sim(inputs):
    """CoreSim path (numerics check without hardware)."""
    from concourse.bass_interp import CoreSim

    nc = _build_module()
    in_maps, obj_id, diam = _prepare_in_maps(**inputs)
    results = []
    for c in range(N_CORES):
        sim = CoreSim(nc)
        for name, val in in_maps[c].items():
            sim.tensor(name)[:] = val
        sim.simulate(check_with_hw=False)
        results.append({"out": np.array(sim.tensor("out"))})
    return _postprocess(results, obj_id, diam)


def kernel(**inputs):
    (pm, t_center, t_depth), _ = run(inputs, trace=False)
    return pm, t_center, t_depth


# revision 50
# speedup vs baseline: 1.2283x; 1.2283x over previous
"""Trainium2 Bass kernel for the pose-estimation loss (pm / t_center / t_depth).

Strategy
--------
pm[n] = mean_p | (pred_R[n]-gt_R[n]) @ obj_points[obj_id[n], p] |_1 / diam[obj_id[n]]

The data-dependent gather obj_points[obj_id] is folded into the matmul:
    Y[(i,n), p] = sum_{o,j} A[(o,j),(i,n)] * B[(o,j), p]
with A[(o,j),(i,n)] = [obj_id[n]==o] * dR[n,i,j]   (24 x 384, built on host)
     B[(o,j), p]    = obj_points[o, p, j]          (24 x 12500 per core)

The bottleneck is draining PSUM through abs+sum.  Only DVE and ACT can read
PSUM, each at 1 elem/lane/cycle (a DVE op may read at most ONE non-scalar
input from PSUM, so no 2-stream tricks).  Measured drain rates: DVE
tensor_reduce(abs) ~115 G elem/s, ACT activation(Abs, accum_out) ~99 G
elem/s; both run flat-out on disjoint PSUM bank pairs, fully fused into
per-instruction accumulator columns.

This version is RAW bass (no TileContext): the whole pipeline is a static
double-buffered schedule synchronized with 8 hand-placed counter
semaphores.  Tile's scheduler allocates ~250 dependency semaphores for
this program and spends ~10us tearing them down inside the measured
window; the manual schedule eliminates that entirely.

Engine program (per core):
  sync ring : A piece | B[512:1536] | B[2560:3312] | out
  scalar ring: B[0:512] | tsite | B[1536:2560]
  Tensor    : per chunk 2 matmuls (q0/q1 -> DVE tile, q2/q3 -> ACT tile),
              4 PE row-group quadrants, K=24, cold-clock 1.2 GHz (HAM never
              engages on this part; even cold the PE outruns the drains).
  Vector    : t_site losses early, then tensor_reduce(abs) per DVE tile,
              final acc-column sum -> out_sb.
  Scalar    : Abs-table warmup, then activation(Abs, accum_out) per ACT tile.

Per core output: out[128, 3] = [pm partial sum, t_center, t_depth].
Host: pm = sum_over_cores(out[:,0]) / 100000 / diam[obj_id].
"""

import os
import sys

import numpy as np

os.environ.setdefault("MYCRO_LOCAL_CACHE", "1")
if "/opt/trn_rl_repo" not in sys.path:
    sys.path.insert(0, "/opt/trn_rl_repo")

# ---- problem constants (hardcoded, must match the reference) ----
N_SAMPLES = 128
NUM_OBJECTS = 8
NUM_POINTS = 100000
N_CORES = 8

PTS_PER_CORE = NUM_POINTS // N_CORES  # 12500
ICHUNKS = 3                           # (i) coordinate chunks of 128 samples
A_COLS = ICHUNKS * 128                # 384

# DVE quadrants (q0, q1) and ACT quadrants (q2, q3): column counts matched
# to the measured drain rates (both engines finish together).
DVE_CHUNKS = [512] * 6 + [152]
DVE_COLS = sum(DVE_CHUNKS)            # 3224
ACT_CHUNKS = [512] * 5 + [466]
ACT_COLS = sum(ACT_CHUNKS)            # 3026
assert 2 * DVE_COLS + 2 * ACT_COLS == PTS_PER_CORE

AB_COLS = A_COLS + DVE_COLS           # 3696 (q2/q3 rows zero-padded at the end)
N_ACC = ICHUNKS * (len(DVE_CHUNKS) + len(ACT_CHUNKS))  # 39 accum columns

# B-piece index covering each 512-col chunk (pieces: 0 = cols 0:512,
# 1 = 512:1536, 2 = 1536:2560, 3 = 2560:3312).
_CHUNK_PIECE = {0: 0, 1: 1, 2: 1, 3: 2, 4: 2, 5: 3, 6: 3}

_CACHE = {}


def _build_module():
    """Build + compile the single-core Bass program (same program on all cores)."""
    if "nc" in _CACHE:
        return _CACHE["nc"]

    import concourse.bass as bass  # noqa: F401  (import registers engines)
    from concourse import bacc, mybir

    f32 = mybir.dt.float32
    bf16 = mybir.dt.bfloat16

    # detect_race_conditions=False: the checker has no notion of same-engine
    # FIFO order (a hardware guarantee) and flags every same-engine
    # write->read chain in a raw-bass program.  Cross-engine ordering is
    # fully covered by the explicit semaphores below.
    nc = bacc.Bacc("TRN2", target_bir_lowering=False, debug=False,
                   detect_race_conditions=False)

    abmat = nc.dram_tensor("abmat", [128, AB_COLS], bf16, kind="ExternalInput").ap()
    tsite = nc.dram_tensor("tsite", [128, 6], f32, kind="ExternalInput").ap()
    out = nc.dram_tensor("out", [128, 3], f32, kind="ExternalOutput").ap()

    ab_sb = nc.alloc_sbuf_tensor("ab_sb", [128, AB_COLS], bf16).ap()
    ts_sb = nc.alloc_sbuf_tensor("ts_sb", [128, 6], f32).ap()
    acc = nc.alloc_sbuf_tensor("acc", [128, N_ACC], f32).ap()
    asum = nc.alloc_sbuf_tensor("asum", [128, 2, 512], bf16).ap()
    out_sb = nc.alloc_sbuf_tensor("out_sb", [128, 3], f32).ap()
    d_sb = nc.alloc_sbuf_tensor("d_sb", [128, 3], f32).ap()

    a_sb = ab_sb[:, 0:A_COLS]
    b_sb = ab_sb[:, A_COLS:]

    # PSUM: DVE tiles in banks 0-3 (two [128,2,512] buffers), ACT tiles in
    # banks 4-7.
    v_ps = [nc.place_psum_tensor(f"v_ps{b}", [128, 2, 512], f32, bank=2 * b).ap()
            for b in range(2)]
    t_ps = [nc.place_psum_tensor(f"t_ps{b}", [128, 2, 512], f32, bank=4 + 2 * b).ap()
            for b in range(2)]

    # Counter semaphores (cleared below before any engine waits on them).
    # One per DMA piece: per-engine completion increments from different
    # pieces on one ring can interleave, so shared-counter thresholds are
    # unsound.
    s_da = nc.alloc_semaphore("s_da")         # A piece (x16)
    s_db = [nc.alloc_semaphore(f"s_db{p}") for p in range(4)]  # B pieces (x16)
    s_dts = nc.alloc_semaphore("s_dts")       # tsite piece (x16)
    s_mmv = nc.alloc_semaphore("s_mmv")       # DVE-path chunks matmul'd
    s_mma = nc.alloc_semaphore("s_mma")       # ACT-path chunks matmul'd
    s_v = nc.alloc_semaphore("s_v")           # DVE tiles drained
    s_a = nc.alloc_semaphore("s_a")           # ACT tiles drained
    s_fin = nc.alloc_semaphore("s_fin")       # ACT accum columns all landed
    s_out = nc.alloc_semaphore("s_out")       # out_sb complete
    s_odma = nc.alloc_semaphore("s_odma")     # out DMA completion (x16)
    all_sems = [s_da, *s_db, s_dts, s_mmv, s_mma, s_v, s_a, s_fin, s_out, s_odma]

    # DMAs first: their completion increments land multi-us later, safely
    # after the gpsimd sem clears below.  All pieces are full 128-partition
    # transfers — partial-partition DMAs generate far more expensive
    # descriptor patterns (measured 2.4x issue time).
    nc.sync.dma_start(out=ab_sb[:, 0:A_COLS],
                      in_=abmat[:, 0:A_COLS]).then_inc(s_da, 16)
    nc.scalar.dma_start(out=ab_sb[:, A_COLS : A_COLS + 512],
                        in_=abmat[:, A_COLS : A_COLS + 512]).then_inc(s_db[0], 16)
    nc.scalar.dma_start(out=ts_sb, in_=tsite).then_inc(s_dts, 16)
    nc.sync.dma_start(out=ab_sb[:, A_COLS + 512 : A_COLS + 1536],
                      in_=abmat[:, A_COLS + 512 : A_COLS + 1536]).then_inc(s_db[1], 16)
    nc.sync.dma_start(out=ab_sb[:, A_COLS + 2560 :],
                      in_=abmat[:, A_COLS + 2560 :]).then_inc(s_db[3], 16)
    nc.scalar.dma_start(out=ab_sb[:, A_COLS + 1536 : A_COLS + 2560],
                        in_=abmat[:, A_COLS + 1536 : A_COLS + 2560]).then_inc(s_db[2], 16)

    # Clear our semaphores (stale values survive across NEFF executions),
    # then barrier so no engine's sem-wait can read a stale value.  The DMA
    # completion increments race this clear only in theory — they land
    # >2us after the clears retire.
    for s in all_sems:
        nc.gpsimd.sem_clear(s)
    nc.all_engine_barrier()

    # No ACT warm-up needed: bacc auto-inserts the Abs ACT_TABLE_LOAD right
    # before the first ACTIVATE in the queue, and the sem wait fuses onto
    # the ACTIVATE itself, so the ~1.3us table load runs while DMAs stream.

    # ---- main pipeline ----
    # Global chunk order: for each i-chunk interleave v0 a0 v1 a1 ... v6.
    jv = 0  # DVE-path chunk counter
    ja = 0  # ACT-path chunk counter
    col = 0
    vcols = []  # acc columns owned by DVE (their sum happens in the final
    acols = []  # reduce regardless; recorded only for clarity)
    for i in range(ICHUNKS):
        ai = slice(i * 128, (i + 1) * 128)
        order = []
        for k in range(len(DVE_CHUNKS)):
            order.append(("v", k))
            if k < len(ACT_CHUNKS):
                order.append(("a", k))
        for kind, k in order:
            off = 512 * k
            s_piece = s_db[_CHUNK_PIECE[k]]
            if kind == "v":
                w = DVE_CHUNKS[k]
                ps = v_ps[jv % 2]
                nc.tensor.wait_ge(s_da, 16)
                nc.tensor.wait_ge(s_piece, 16)
                if jv >= 2:
                    nc.tensor.wait_ge(s_v, jv - 1)
                nc.tensor.matmul(
                    ps[:, 0, 0:w], lhsT=a_sb[0:24, ai],
                    rhs=b_sb[0:24, off : off + w],
                    start=True, stop=True, tile_position=(0, 0),
                )
                nc.tensor.matmul(
                    ps[:, 1, 0:w], lhsT=a_sb[32:56, ai],
                    rhs=b_sb[32:56, off : off + w],
                    start=True, stop=True, tile_position=(32, 0),
                ).then_inc(s_mmv, 1)
                nc.vector.wait_ge(s_mmv, jv + 1)
                nc.vector.tensor_reduce(
                    out=acc[:, col : col + 1], in_=ps[:, :, 0:w],
                    axis=mybir.AxisListType.XY, op=mybir.AluOpType.add,
                    apply_absolute_value=True,
                ).then_inc(s_v, 1)
                jv += 1
                vcols.append(col)
            else:
                w = ACT_CHUNKS[k]
                ps = t_ps[ja % 2]
                nc.tensor.wait_ge(s_da, 16)
                nc.tensor.wait_ge(s_piece, 16)
                if ja >= 2:
                    nc.tensor.wait_ge(s_a, ja - 1)
                nc.tensor.matmul(
                    ps[:, 0, 0:w], lhsT=a_sb[64:88, ai],
                    rhs=b_sb[64:88, off : off + w],
                    start=True, stop=True, tile_position=(64, 0),
                )
                nc.tensor.matmul(
                    ps[:, 1, 0:w], lhsT=a_sb[96:120, ai],
                    rhs=b_sb[96:120, off : off + w],
                    start=True, stop=True, tile_position=(96, 0),
                ).then_inc(s_mma, 1)
                nc.scalar.wait_ge(s_mma, ja + 1)
                nc.scalar.activation(
                    out=asum[:, :, 0:w], in_=ps[:, :, 0:w],
                    func=mybir.ActivationFunctionType.Abs,
                    accum_out=acc[:, col : col + 1],
                ).then_inc(s_a, 1)
                ja += 1
                acols.append(col)
            col += 1
    assert jv == ICHUNKS * len(DVE_CHUNKS) and ja == ICHUNKS * len(ACT_CHUNKS)

    # ACT signals its accumulator columns are all written (queue-ordered
    # after the last ACTIVATION_READ_ACCUMULATOR).
    nc.scalar.nop(nofuse=True).then_inc(s_fin, 1)

    # t_site losses, placed after the drain loop so they never delay the
    # first chunk drains; they run in the shadow of ACT's remaining chunks:
    # d = gt - pred; t_center = |d0|+|d1|; t_depth = |d2|.
    nc.vector.wait_ge(s_dts, 16)
    nc.vector.tensor_sub(d_sb, ts_sb[:, 0:3], ts_sb[:, 3:6])
    # Explicit pipe drain: raw bass does not auto-insert the DVE DRAIN, and
    # the next op would read d_sb while the sub's writes are still in the
    # 8-stage pipe.
    nc.vector.drain()
    nc.vector.tensor_reduce(
        out=out_sb[:, 1:2], in_=d_sb[:, 0:2], axis=mybir.AxisListType.X,
        op=mybir.AluOpType.add, apply_absolute_value=True,
    )
    nc.vector.tensor_reduce(
        out=out_sb[:, 2:3], in_=d_sb[:, 2:3], axis=mybir.AxisListType.X,
        op=mybir.AluOpType.add, apply_absolute_value=True,
    )

    # Final: pm partial = sum of all accumulator columns.  Drain first: the
    # last DVE chunk-reduce wrote its acc column on this same engine and raw
    # bass does not auto-insert the DVE pipe DRAIN.
    nc.vector.wait_ge(s_fin, 1)
    nc.vector.drain()
    nc.vector.tensor_reduce(
        out=out_sb[:, 0:1], in_=acc[:, 0:col], axis=mybir.AxisListType.X,
        op=mybir.AluOpType.add,
    ).then_inc(s_out, 1)

    # No final barrier: each engine halts when its queue ends, so the NEFF
    # packager's per-engine postamble (a ~6us semaphore sweep) overlaps the
    # other engines' remaining work instead of starting after a global
    # barrier.  The runtime fences the in-flight output DMA at NEFF end.
    nc.sync.wait_ge(s_out, 1)
    nc.sync.dma_start(out=out, in_=out_sb).then_inc(s_odma, 16)

    nc.compile()
    _CACHE["nc"] = nc
    return nc


def _prepare_in_maps(obj_id, gt_cam_R_m2c, pred_cam_R_m2c, gt_cam_t_m2c_site,
                     pred_cam_t_m2c_site, obj_points, obj_diameters):
    obj_id = np.asarray(obj_id).astype(np.int64)
    dR = (np.asarray(pred_cam_R_m2c, np.float32)
          - np.asarray(gt_cam_R_m2c, np.float32))          # [N, 3, 3] (i, j)
    pts = np.asarray(obj_points, np.float32)               # [8, P, 3]

    import ml_dtypes

    # A[(o,j), (i,n)] = [obj_id[n]==o] * dR[n, i, j]
    afull = np.zeros((NUM_OBJECTS, 3, 3, N_SAMPLES), np.float32)  # [o, j, i, n]
    afull[obj_id, :, :, np.arange(N_SAMPLES)] = dR.transpose(0, 2, 1)  # [n, j, i]
    a24 = afull.reshape(NUM_OBJECTS * 3, 3 * N_SAMPLES)    # rows (o,j), cols i*128+n

    # B rows (o,j), cols p
    b24 = pts.transpose(0, 2, 1).reshape(NUM_OBJECTS * 3, NUM_POINTS)

    ts_host = np.concatenate(
        [np.asarray(gt_cam_t_m2c_site, np.float32),
         np.asarray(pred_cam_t_m2c_site, np.float32)], axis=1)  # [128, 6]

    quad_cols = [DVE_COLS, DVE_COLS, ACT_COLS, ACT_COLS]
    in_maps = []
    for c in range(N_CORES):
        cols = b24[:, c * PTS_PER_CORE : (c + 1) * PTS_PER_CORE]
        ab = np.zeros((128, AB_COLS), np.float32)
        off = 0
        for g in range(4):
            w = quad_cols[g]
            ab[32 * g : 32 * g + 24, 0:A_COLS] = a24
            ab[32 * g : 32 * g + 24, A_COLS : A_COLS + w] = cols[:, off : off + w]
            off += w
        in_maps.append({
            "abmat": np.ascontiguousarray(ab).astype(ml_dtypes.bfloat16),
            "tsite": ts_host,
        })
    return in_maps, obj_id, np.asarray(obj_diameters, np.float32)


def _postprocess(results, obj_id, obj_diameters):
    pm_sum = np.zeros(N_SAMPLES, np.float64)
    for c in range(N_CORES):
        pm_sum += results[c]["out"][:, 0].astype(np.float64)
    pm = (pm_sum / NUM_POINTS / obj_diameters[obj_id].astype(np.float64)).astype(
        np.float32)
    t_center = results[0]["out"][:, 1].astype(np.float32)
    t_depth = results[0]["out"][:, 2].astype(np.float32)
    return pm, t_center, t_depth


def run(inputs, trace=False):
    """Run on the 8 NeuronCores. Returns ((pm, t_center, t_depth), BassKernelResults)."""
    from concourse.bass_utils import run_bass_kernel_spmd

    nc = _build_module()
    in_maps, obj_id, diam = _prepare_in_maps(**inputs)
    res = run_bass_kernel_spmd(nc, in_maps, list(range(N_CORES)), trace=trace)
    return _postprocess(res.results, obj_id, diam), res


def run_sim(inputs):
    """CoreSim path (numerics check without hardware)."""
    from concourse.bass_interp import CoreSim

    nc = _build_module()
    in_maps, obj_id, diam = _prepare_in_maps(**inputs)
    results = []
    for c in range(N_CORES):
        sim = CoreSim(nc)
        for name, val in in_maps[c].items():
            sim.tensor(name)[:] = val
        sim.simulate(check_with_hw=False)
        results.append({"out": np.array(sim.tensor("out"))})
    return _postprocess(results, obj_id, diam)


def kernel(**inputs):
    (pm, t_center, t_depth), _ = run(inputs, trace=False)
    return pm, t_center, t_depth
